# revision 5
# baseline (speedup 1.0000x reference)
"""Embedding lookup (weight[indices]) on 8 TRN2 NeuronCores.

Table replicated per core (bf16), indices sharded 8 ways.  The naive
per-row indirect DMA is SWDGE-instruction-bound (~1 us per 128 rows), so
instead each core runs a sorted two-step pipeline built from the bulk
SWDGE instructions (0.34 ns/descriptor):

  host:   sort the core's tokens by (out_window, table_window) into
          fixed-capacity groups; emit int16 window-local gather indices
          and int16 out-window-local scatter indices (pads gather a
          duplicate row and scatter into per-window trash blocks).
  device: pre-zero out_buf (scatter is CCE-add); per 16384-token SBUF
          tile: windowed dma_gather instructions fill the tile in stream
          order, then dma_scatter_add instructions write each row to its
          original output position.

The instruction structure depends only on shapes (fixed group capacity),
so one compiled NEFF serves all cores and calls.  bf16 halves HBM
traffic; the rel-err of bf16 rounding (~0.4%) is well inside the 2e-2
gate.
"""

import numpy as np
import ml_dtypes

P = 128
D = 128
NUM_EMB = 1_000_000
N_CORES = 8

TW_ROWS = 32768    # table window (int16 gather index range)
OW_ROWS = 25600    # out window (int16 scatter index range, < 32768)
CAP = 1024         # fixed tokens per (ow, tw) group; avg is ~826
TILE_TOK = 16384   # tokens per SBUF tile
TRASH = P          # trash rows appended to each out window (pad target)
N_QUEUES = 4
DT_NAME = "bfloat16"

_CACHE = {}


class Plan:
    """Static structure parameters + host index routing."""

    def __init__(self, num_emb, d, per_core, tw_rows, ow_rows, cap, tile_tok,
                 trash_rows=P):
        assert cap % 16 == 0 and tile_tok % cap == 0
        self.num_emb, self.d, self.per_core = num_emb, d, per_core
        self.tw_rows, self.ow_rows, self.cap, self.tile_tok = tw_rows, ow_rows, cap, tile_tok
        self.trash_rows = trash_rows
        self.n_tw = -(-num_emb // tw_rows)
        self.n_ow = -(-per_core // ow_rows)
        assert ow_rows % P == 0 and per_core % ow_rows == 0
        self.n_groups = self.n_ow * self.n_tw
        self.groups_per_tile = tile_tok // cap
        self.n_tiles = -(-self.n_groups // self.groups_per_tile)
        self.ow_stride = ow_rows + trash_rows
        self.out_rows = self.n_ow * self.ow_stride
        assert self.out_rows % P == 0
        self.tiles = []
        for t in range(self.n_tiles):
            g0 = t * self.groups_per_tile
            g1 = min(g0 + self.groups_per_tile, self.n_groups)
            gathers = [((g - g0) * cap, g % self.n_tw) for g in range(g0, g1)]
            runs = []
            rs = g0
            for g in range(g0 + 1, g1 + 1):
                if g == g1 or (g // self.n_tw) != (rs // self.n_tw):
                    # split runs so one scatter stays within the SWDGE
                    # descriptor ring (~1024 descs/engine; tx pushes
                    # 2*tokens/16 per engine => cap tokens at 4096)
                    tok0, ntok = (rs - g0) * cap, (g - rs) * cap
                    ow = rs // self.n_tw
                    MAXTOK = 4096
                    for off in range(0, ntok, MAXTOK):
                        runs.append((tok0 + off, min(MAXTOK, ntok - off), ow))
                    rs = g
            self.tiles.append((gathers, runs))

    def route(self, idx):
        n = self.per_core
        assert idx.shape == (n,)
        pos = np.arange(n, dtype=np.int64)
        ow = pos // self.ow_rows
        tw = idx // self.tw_rows
        key = ow * self.n_tw + tw
        order = np.argsort(key, kind="stable")
        skey = key[order]
        counts = np.bincount(skey, minlength=self.n_groups)
        if counts.max() > self.cap:
            raise OverflowError(f"group count {counts.max()} > cap {self.cap}")
        total = self.n_tiles * self.tile_tok
        g_local = np.zeros(total, dtype=np.int16)
        s_local = np.zeros(total, dtype=np.int16)
        starts = np.concatenate([[0], np.cumsum(counts)[:-1]])
        for g in range(self.n_groups):
            c = counts[g]
            o = g * self.cap
            seg = order[starts[g] : starts[g] + c]
            tw_g, ow_g = g % self.n_tw, g // self.n_tw
            gl = (idx[seg] - tw_g * self.tw_rows).astype(np.int16)
            sl = (pos[seg] - ow_g * self.ow_rows).astype(np.int16)
            g_local[o : o + c] = gl
            s_local[o : o + c] = sl
            pad = self.cap - c
            if pad:
                g_local[o + c : o + self.cap] = gl[0] if c else 0
                s_local[o + c : o + self.cap] = (
                    self.ow_rows + (np.arange(pad) % self.trash_rows)
                ).astype(np.int16)
        def wrap(a):
            a = a.reshape(self.n_tiles, self.tile_tok // 16, 16)
            a = np.swapaxes(a, 1, 2)
            return np.ascontiguousarray(np.tile(a, (1, 8, 1)))
        return wrap(g_local), wrap(s_local)

    def extract(self, out_buf):
        a = out_buf.reshape(self.n_ow, self.ow_stride, self.d)
        return a[:, : self.ow_rows].reshape(-1, self.d)[: self.per_core]


def _build_bass(plan, dt_name, n_queues):
    import concourse.bacc as bacc
    import concourse.mybir as mybir
    import concourse.tile as tile

    key = (plan.num_emb, plan.d, plan.per_core, plan.cap, plan.tile_tok,
           dt_name, n_queues)
    if key in _CACHE:
        return _CACHE[key]

    dt = getattr(mybir.dt, dt_name)
    d = plan.d
    nc = bacc.Bacc(
        "TRN2",
        target_bir_lowering=False,
        debug=False,
        num_devices=N_CORES,
        num_swdge_queues=max(1, n_queues),
    )
    weight = nc.dram_tensor("weight", [plan.num_emb, d], dt, kind="ExternalInput")
    gidx = nc.dram_tensor(
        "gidx", [plan.n_tiles, P, plan.tile_tok // 16], mybir.dt.int16,
        kind="ExternalInput",
    )
    sidx = nc.dram_tensor(
        "sidx", [plan.n_tiles, P, plan.tile_tok // 16], mybir.dt.int16,
        kind="ExternalInput",
    )
    out = nc.dram_tensor("out", [plan.out_rows, d], dt, kind="ExternalOutput")

    blk_per_group = plan.cap // P
    with tile.TileContext(nc) as tc:
        with (
            tc.tile_pool(name="zp", bufs=1) as zp,
            tc.tile_pool(name="gip", bufs=2) as gip,
            tc.tile_pool(name="sip", bufs=2) as sip,
            tc.tile_pool(name="data", bufs=3) as datap,
        ):
            ZCOLS = 4096
            ztile = zp.tile([P, ZCOLS], dt)
            nc.vector.memset(ztile[:], 0.0)
            out_flat = out[:].rearrange("(p n) d -> p (n d)", p=P)
            zlen = plan.out_rows // P * d
            off = 0
            while off < zlen:
                c = min(ZCOLS, zlen - off)
                nc.sync.dma_start(out_flat[:, off : off + c], ztile[:, :c])
                off += c

            q = 0
            for t, (gathers, runs) in enumerate(plan.tiles):
                git = gip.tile([P, plan.tile_tok // 16], mybir.dt.int16)
                nc.sync.dma_start(git[:], gidx[t, :, :])
                sit = sip.tile([P, plan.tile_tok // 16], mybir.dt.int16)
                nc.sync.dma_start(sit[:], sidx[t, :, :])
                dtile = datap.tile([P, (plan.tile_tok // P) * d], dt)
                d3 = dtile[:].rearrange("p (b d) -> p b d", d=d)
                for tok_off, tw in gathers:
                    b0 = tok_off // P
                    base = tw * plan.tw_rows
                    rows = min(plan.tw_rows, plan.num_emb - base)
                    nc.gpsimd.dma_gather(
                        out_ap=d3[:, b0 : b0 + blk_per_group, :],
                        in_ap=weight[base : base + rows, :],
                        idxs_ap=git[:, tok_off // 16 : (tok_off + plan.cap) // 16],
                        num_idxs=plan.cap,
                        num_idxs_reg=plan.cap,
                        elem_size=d,
                        queue_num=q % max(1, n_queues),
                    )
                    q += 1
                for tok_off, ntok, ow in runs:
                    b0 = tok_off // P
                    base = ow * plan.ow_stride
                    nc.gpsimd.dma_scatter_add(
                        out_ap=out[base : base + plan.ow_stride, :],
                        in_ap=d3[:, b0 : b0 + ntok // P, :],
                        idxs_ap=sit[:, tok_off // 16 : (tok_off + ntok) // 16],
                        num_idxs=ntok,
                        num_idxs_reg=ntok,
                        elem_size=d,
                        queue_num=q % max(1, n_queues),
                    )
                    q += 1
    nc.compile()
    _CACHE[key] = nc
    return nc


def run_sharded(indices: np.ndarray, weight: np.ndarray, trace: bool = False):
    from concourse.bass_utils import run_bass_kernel_spmd

    dt_np = ml_dtypes.bfloat16 if DT_NAME == "bfloat16" else np.float32
    idx_flat = np.ascontiguousarray(indices.reshape(-1).astype(np.int64))
    n_idx = idx_flat.shape[0]
    per_core = n_idx // N_CORES
    assert n_idx == per_core * N_CORES

    plan = Plan(NUM_EMB, D, per_core, TW_ROWS, OW_ROWS, CAP, TILE_TOK)
    nc = _build_bass(plan, DT_NAME, N_QUEUES)

    w = np.ascontiguousarray(weight.astype(dt_np))
    in_maps = []
    for c in range(N_CORES):
        gidx, sidx = plan.route(idx_flat[c * per_core : (c + 1) * per_core])
        in_maps.append({"weight": w, "gidx": gidx, "sidx": sidx})
    res = run_bass_kernel_spmd(
        nc, in_maps, core_ids=list(range(N_CORES)), trace=trace
    )
    full = np.concatenate(
        [plan.extract(r["out"]) for r in res.results], axis=0
    ).astype(np.float32)
    return full.reshape(indices.shape + (D,)), res


def kernel(indices: np.ndarray, weight: np.ndarray) -> np.ndarray:
    full, _ = run_sharded(indices, weight, trace=False)
    return full
